# revision 29
# baseline (speedup 1.0000x reference)
"""CapsuleLayer (single routing iteration) Trainium2 kernel.

Math (per batch element b of x: (B=64, NU=32, IC=256, US=128) fp32):
  - torch-style reshape of x[b] to (IC, NU, US): row i of the flat
    (256, 4096) view is x[b].flat[i*4096:(i+1)*4096].
  - s[j]   = (1/256) * sum_i flat[i, j]          (j = n*128+u, 4096 outputs)
  - msq[n] = sum_u s[n,u]^2
  - out[n,u] = msq/(1+msq) * s[n,u]/(sqrt(msq)+1e-5)
             ~ s[n,u] * sqrt(msq)/(1+msq)        (1e-5 dropped: ~1.4e-5 rel)

Sharding: pure batch data-parallel over 8 NeuronCores (8 batches/core).

Precision: the host casts x to bfloat16 before feeding the device —
halving HBM traffic, the dominant cost. The averaging over 256 ic
shrinks the rounding noise ~16x, so output rel-err is ~1.6e-3 against
the 2e-2 tolerance. PSUM accumulation stays fp32.

Per-core pipeline (~16 MiB HBM reads per core; one HWDGE queue alone
saturates ~425-430 GB/s, so the two rings exist to hide per-DMA
completion bubbles, not for bandwidth; post-bf16 the PE's ~373 ns/matmul
cadence is the limiter):
  - per batch: h0 half (1 MiB) on the SP ring, h1 half on the ACT ring.
    Batch 0 is chunked so both rings engage immediately; batch 7 is
    loaded as 16x 128 KiB chunks alternating rings, group-A chunks
    (k0-3, both halves) first.
  - 16 bf16 matmuls per batch reduce ic via the PE into one PSUM
    (8, 512) fp32 accumulation group (lhsT column k holds 1/256). Batch 7
    uses two (4, 512) row-group accumulations: group A closes ~5 us
    before the last byte, so its whole squash+store hides under group
    B's loads; only B's (4,512) squash trails the final byte.
  - squash: one scalar Square -> DVE tensor_reduce -> scalar Sqrt ->
    DVE add/recip/mul -> DVE broadcast multiply (the reference's +1e-5
    in the denominator is dropped: ~1.4e-5 relative, tolerance 2e-2).
    Deferred ONE batch behind the loads so squash never sits between
    two load triggers in an engine's HWDGE FIFO (in-order dispatch per
    engine; the v1 kernel lost ~5 us of HBM time to that).
  - outputs accumulate in one SBUF tile; batches 0-6 stored in one
    112 KiB DMA issued while batch 7 still streams (write receipt
    hidden), batch 7 as two 8 KiB stores right after its squashes.

"""

import ml_dtypes
import numpy as np

import concourse.bass as bass
import concourse.bacc as bacc
import concourse.mybir as mybir
import concourse.tile as tile
from concourse.bass_utils import run_bass_kernel_spmd

B, NU, IC, US = 64, 32, 256, 128
N_CORES = 8
PB = B // N_CORES            # batches per core
F = NU * US                  # 4096 outputs per batch
HALVES = IC // 128           # 2 partition-halves of the ic axis
NBANK = F // 512             # 8 matmul chunks (one PSUM partition row each)
NQ = F // NBANK // 128       # 4 u-groups per PSUM partition row


def build_bass(pb=PB):
    PB = pb
    nc = bacc.Bacc("TRN2", target_bir_lowering=False, debug=False)

    mm_dt = mybir.dt.bfloat16

    x = nc.dram_tensor("x", [PB, HALVES, 128, F], mm_dt,
                       kind="ExternalInput")
    w = nc.dram_tensor("w", [128, NBANK, NBANK], mm_dt,
                       kind="ExternalInput")
    y = nc.dram_tensor("y", [NBANK, PB, 512], mybir.dt.float32,
                       kind="ExternalOutput")

    with tile.TileContext(nc) as tc:
        with (
            tc.tile_pool(name="const", bufs=1) as const_pool,
            tc.tile_pool(name="acc", bufs=6) as acc_pool,
            tc.tile_pool(name="psum", bufs=4, space="PSUM") as psum_pool,
            tc.tile_pool(name="sq", bufs=2) as sq_pool,
            tc.tile_pool(name="stats", bufs=4) as stats_pool,
            tc.tile_pool(name="outp", bufs=1) as out_pool,
        ):
            # Selection weights: sel[:, k, j] = 1/256 iff j == k.
            # (loaded from DRAM; SWDGE keeps
            # it off the two load rings)
            sel = const_pool.tile([128, NBANK, NBANK], mm_dt)
            nc.gpsimd.dma_start(out=sel[:], in_=w[:])

            # All 8 batches' squashed outputs land here; one store at the end.
            outt = out_pool.tile([NBANK, PB, 512], mybir.dt.float32)

            state = {}
            rings = (nc.sync, nc.scalar)

            def load_full(b):
                th = [acc_pool.tile([128, F], mm_dt, tag="acc",
                                    name=f"t{b}h{h}")
                      for h in range(HALVES)]
                for h in range(HALVES):
                    rings[h].dma_start(out=th[h][:], in_=x[b, h])
                return th

            def load_tail_chunks(b, th, pieces, i0=0):
                # pieces: (h, col0, col1); ring alternates with sequence idx
                for i, (h, c0, c1) in enumerate(pieces):
                    rings[(i0 + i) % 2].dma_start(
                        out=th[h][:, c0:c1],
                        in_=x[b, h, :, c0:c1])

            def mms(b, th):
                # k-major: consecutive matmuls share the same stationary
                # weights, letting codegen skip redundant LDWEIGHTS (the
                # PE cadence, not DMA, limits the bf16 kernel).
                ps = psum_pool.tile([NBANK, 512], mybir.dt.float32, tag="ps")
                for k in range(NBANK):
                    for h in range(HALVES):
                        nc.tensor.matmul(
                            ps[:, :],
                            sel[:, k, :],
                            th[h][:, k * 512:(k + 1) * 512],
                            start=(k == 0 and h == 0),
                            stop=(k == NBANK - 1 and h == HALVES - 1),
                        )
                state[b] = ps

            def squash(ps_ap, nr, nq, out_ap, tg):
                # squash an (nr, nq*128) PSUM block into out_ap
                sq = sq_pool.tile([nr, nq * 128], mybir.dt.float32,
                                  tag="sq" + tg, name="sq" + tg)
                nc.scalar.activation(out=sq[:], in_=ps_ap,
                                     func=mybir.ActivationFunctionType.Square)
                msq = stats_pool.tile([nr, nq], mybir.dt.float32,
                                      tag="msq" + tg, name="msq" + tg)
                nc.vector.tensor_reduce(
                    out=msq[:],
                    in_=sq[:].rearrange("p (q u) -> p q u", q=nq),
                    axis=mybir.AxisListType.X,
                    op=mybir.AluOpType.add)
                mag = stats_pool.tile([nr, nq], mybir.dt.float32,
                                      tag="mag" + tg, name="mag" + tg)
                nc.scalar.activation(out=mag[:], in_=msq[:],
                                     func=mybir.ActivationFunctionType.Sqrt)
                # fac = mag * 1/(1 + msq)  (== msq/((1+msq)sqrt(msq)))
                t1 = stats_pool.tile([nr, nq], mybir.dt.float32,
                                     tag="t1" + tg, name="t1" + tg)
                nc.vector.tensor_scalar_add(t1[:], msq[:], 1.0)
                rec = stats_pool.tile([nr, nq], mybir.dt.float32,
                                      tag="rec" + tg, name="rec" + tg)
                nc.vector.reciprocal(rec[:], t1[:])
                fac = stats_pool.tile([nr, nq], mybir.dt.float32,
                                      tag="fac" + tg, name="fac" + tg)
                nc.vector.tensor_mul(fac[:], mag[:], rec[:])
                fap = fac[:]
                fb = bass.AP(tensor=fap.tensor, offset=fap.offset,
                             ap=[fap.ap[0], fap.ap[1], [0, 128]])
                nc.vector.tensor_tensor(
                    out_ap.rearrange("p (q u) -> p q u", q=nq),
                    ps_ap.rearrange("p (q u) -> p q u", q=nq),
                    fb, mybir.AluOpType.mult)

            def finish(b):
                squash(state.pop(b)[:], NBANK, NQ, outt[:, b, :], "")

            for b in range(PB - 1):
                if b == 0:
                    # Ramp shaping. A DMA trigger occupies its engine
                    # ~0.6-1 us per MiB of descriptor-gen, so tiny first
                    # chunks get the first bytes moving sooner; h1's first
                    # 256 KiB rides the SP ring because the ACT ring's
                    # first descriptors queue behind its activation-table
                    # load (~1.3 us) and would otherwise idle the stream.
                    th = [acc_pool.tile([128, F], mm_dt, tag="acc",
                                        name=f"t{b}h{h}")
                          for h in range(HALVES)]
                    parts = [(0, 0, 512), (1, 0, 512), (1, 512, 1536),
                             (0, 512, 1536), (1, 1536, 2560),
                             (0, 1536, 2560), (1, 2560, F), (0, 2560, F)]
                    for i, (h, c0, c1) in enumerate(parts):
                        ring = rings[0] if (h == 0 or i == 1) else rings[1]
                        ring.dma_start(out=th[h][:, c0:c1],
                                       in_=x[b, h, :, c0:c1])
                else:
                    th = load_full(b)
                if b >= 1:
                    finish(b - 1)
                mms(b, th)

            # Last batch: fine-grained chunks on both rings. PSUM splits in
            # two row-groups; group A's data (chunks k0-3, both halves) is
            # loaded FIRST so its whole squash hides under group B's loads.
            # B's final 512-col chunk is split by column-halves so the last
            # squash runs as two pipelined half-chains.
            b = PB - 1
            th = [acc_pool.tile([128, F], mm_dt, tag="acc", name=f"t{b}h{h}")
                  for h in range(HALVES)]
            pA = [(h, k * 512, (k + 1) * 512)
                  for h in range(HALVES) for k in range(4)]
            pB = [(h, k * 512, (k + 1) * 512)
                  for h in range(HALVES) for k in range(4, NBANK)]
            load_tail_chunks(b, th, pA)
            finish(b - 1)
            load_tail_chunks(b, th, pB, i0=1)
            # Batches 0-6 stored early: the HBM-write receipt hides under
            # the remaining loads instead of the kernel tail.
            nc.sync.dma_start(out=y[:, :PB - 1], in_=outt[:, :PB - 1])

            psA = psum_pool.tile([4, 512], mybir.dt.float32, tag="psA",
                                 bufs=1)
            psB = psum_pool.tile([4, 512], mybir.dt.float32, tag="psB",
                                 bufs=1)
            for h in range(HALVES):
                for k in range(4):
                    nc.tensor.matmul(
                        psA[:, :], sel[:, k, 0:4],
                        th[h][:, k * 512:(k + 1) * 512],
                        start=(h == 0 and k == 0),
                        stop=(h == HALVES - 1 and k == 3))
            for h in range(HALVES):
                for k in range(4, NBANK):
                    nc.tensor.matmul(
                        psB[:, :], sel[:, k, 4:8],
                        th[h][:, k * 512:(k + 1) * 512],
                        start=(h == 0 and k == 4),
                        stop=(h == HALVES - 1 and k == NBANK - 1))
            squash(psA[:], 4, NQ, outt[:4, b, :], "A")
            nc.sync.dma_start(out=y[:4, b], in_=outt[:4, b])
            outb = out_pool.tile([4, 512], mybir.dt.float32)
            squash(psB[:], 4, NQ, outb[:], "B")
            nc.sync.dma_start(out=y[4:, b], in_=outb[:])

    nc.compile()
    return nc


_NC_CACHE = {}


def _get_nc():
    if "nc" not in _NC_CACHE:
        _NC_CACHE["nc"] = build_bass()
    return _NC_CACHE["nc"]


def kernel(x, **run_kwargs):
    x = np.ascontiguousarray(np.asarray(x, dtype=np.float32))
    assert x.shape == (B, NU, IC, US), x.shape

    nc = _get_nc()
    xs = x.reshape(N_CORES, PB, HALVES, 128, F)
    w = np.zeros((128, NBANK, NBANK), dtype=np.float32)
    for k in range(NBANK):
        w[:, k, k] = 1.0 / IC
    wb = w.astype(ml_dtypes.bfloat16)
    in_maps = [{"x": np.ascontiguousarray(xs[c]).astype(ml_dtypes.bfloat16),
                "w": wb}
               for c in range(N_CORES)]
    res = run_bass_kernel_spmd(nc, in_maps, core_ids=list(range(N_CORES)),
                               **run_kwargs)
    # y is (NBANK, PB, 512) per core; batch-major reshape on the host.
    out = np.stack([r["y"].transpose(1, 0, 2) for r in res.results], axis=0)
    out = out.reshape(B, NU, US, 1)
    if run_kwargs:
        kernel.last_results = res
    return out


# revision 31
# speedup vs baseline: 1.0220x; 1.0220x over previous
"""CapsuleLayer (single routing iteration) Trainium2 kernel.

Math (per batch element b of x: (B=64, NU=32, IC=256, US=128) fp32):
  - torch-style reshape of x[b] to (IC, NU, US): row i of the flat
    (256, 4096) view is x[b].flat[i*4096:(i+1)*4096].
  - s[j]   = (1/256) * sum_i flat[i, j]          (j = n*128+u, 4096 outputs)
  - msq[n] = sum_u s[n,u]^2
  - out[n,u] = msq/(1+msq) * s[n,u]/(sqrt(msq)+1e-5)
             ~ s[n,u] * sqrt(msq)/(1+msq)        (1e-5 dropped: ~1.4e-5 rel)

Sharding: pure batch data-parallel over 8 NeuronCores (8 batches/core).

Precision: the host casts x to bfloat16 before feeding the device —
halving HBM traffic, the dominant cost. The averaging over 256 ic
shrinks the rounding noise ~16x, so output rel-err is ~1.6e-3 against
the 2e-2 tolerance. PSUM accumulation stays fp32.

Per-core pipeline (~16 MiB HBM reads per core; one HWDGE queue alone
saturates ~425-430 GB/s, so the two rings exist to hide per-DMA
completion bubbles, not for bandwidth; post-bf16 the PE's ~373 ns/matmul
cadence is the limiter):
  - per batch: h0 half (1 MiB) on the SP ring, h1 half on the ACT ring.
    Batch 0 is chunked so both rings engage immediately; batch 7 is
    loaded as 16x 128 KiB chunks alternating rings, group-A chunks
    (k0-3, both halves) first.
  - 16 bf16 matmuls per batch reduce ic via the PE into one PSUM
    (8, 512) fp32 accumulation group (lhsT column k holds 1/256). Batch 7
    uses two (4, 512) row-group accumulations: group A closes ~5 us
    before the last byte, so its whole squash+store hides under group
    B's loads; only B's (4,512) squash trails the final byte.
  - squash: one scalar Square -> DVE tensor_reduce -> scalar Sqrt ->
    DVE add/recip/mul -> DVE broadcast multiply (the reference's +1e-5
    in the denominator is dropped: ~1.4e-5 relative, tolerance 2e-2).
    Deferred ONE batch behind the loads so squash never sits between
    two load triggers in an engine's HWDGE FIFO (in-order dispatch per
    engine; the v1 kernel lost ~5 us of HBM time to that).
  - outputs accumulate in one SBUF tile; batches 0-6 stored in one
    112 KiB DMA issued while batch 7 still streams (write receipt
    hidden), batch 7 as two 8 KiB stores right after its squashes.

"""

import ml_dtypes
import numpy as np

import concourse.bass as bass
import concourse.bacc as bacc
import concourse.mybir as mybir
import concourse.tile as tile
from concourse.bass_utils import run_bass_kernel_spmd

B, NU, IC, US = 64, 32, 256, 128
N_CORES = 8
PB = B // N_CORES            # batches per core
F = NU * US                  # 4096 outputs per batch
HALVES = IC // 128           # 2 partition-halves of the ic axis
NBANK = F // 512             # 8 matmul chunks (one PSUM partition row each)
NQ = F // NBANK // 128       # 4 u-groups per PSUM partition row


def build_bass(pb=PB):
    PB = pb
    nc = bacc.Bacc("TRN2", target_bir_lowering=False, debug=False)

    mm_dt = mybir.dt.bfloat16

    x = nc.dram_tensor("x", [PB, HALVES, 128, F], mm_dt,
                       kind="ExternalInput")
    w = nc.dram_tensor("w", [128, NBANK, NBANK], mm_dt,
                       kind="ExternalInput")
    y = nc.dram_tensor("y", [NBANK, PB, 512], mybir.dt.float32,
                       kind="ExternalOutput")

    with tile.TileContext(nc) as tc:
        with (
            tc.tile_pool(name="const", bufs=1) as const_pool,
            tc.tile_pool(name="acc", bufs=6) as acc_pool,
            tc.tile_pool(name="psum", bufs=4, space="PSUM") as psum_pool,
            tc.tile_pool(name="sq", bufs=2) as sq_pool,
            tc.tile_pool(name="stats", bufs=4) as stats_pool,
            tc.tile_pool(name="outp", bufs=1) as out_pool,
        ):
            # Selection weights: sel[:, k, j] = 1/256 iff j == k. First on
            # the SP ring: the PE is saturated end-to-end, so the first
            # LDWEIGHTS (gated on sel) sets the finish time — HWDGE gets
            # sel resident ~2 us sooner than the SWDGE path did.
            sel = const_pool.tile([128, NBANK, NBANK], mm_dt)
            nc.sync.dma_start(out=sel[:], in_=w[:])

            # All 8 batches' squashed outputs land here; one store at the end.
            outt = out_pool.tile([NBANK, PB, 512], mybir.dt.float32)

            state = {}
            rings = (nc.sync, nc.scalar)

            def load_full(b):
                th = [acc_pool.tile([128, F], mm_dt, tag="acc",
                                    name=f"t{b}h{h}")
                      for h in range(HALVES)]
                for h in range(HALVES):
                    rings[h].dma_start(out=th[h][:], in_=x[b, h])
                return th

            def load_tail_chunks(b, th, pieces, i0=0):
                # pieces: (h, col0, col1); ring alternates with sequence idx
                for i, (h, c0, c1) in enumerate(pieces):
                    rings[(i0 + i) % 2].dma_start(
                        out=th[h][:, c0:c1],
                        in_=x[b, h, :, c0:c1])

            def mms(b, th):
                ps = psum_pool.tile([NBANK, 512], mybir.dt.float32, tag="ps")
                for h in range(HALVES):
                    for k in range(NBANK):
                        nc.tensor.matmul(
                            ps[:, :],
                            sel[:, k, :],
                            th[h][:, k * 512:(k + 1) * 512],
                            start=(h == 0 and k == 0),
                            stop=(h == HALVES - 1 and k == NBANK - 1),
                        )
                state[b] = ps

            def squash(ps_ap, nr, nq, out_ap, tg):
                # squash an (nr, nq*128) PSUM block into out_ap
                sq = sq_pool.tile([nr, nq * 128], mybir.dt.float32,
                                  tag="sq" + tg, name="sq" + tg)
                nc.scalar.activation(out=sq[:], in_=ps_ap,
                                     func=mybir.ActivationFunctionType.Square)
                msq = stats_pool.tile([nr, nq], mybir.dt.float32,
                                      tag="msq" + tg, name="msq" + tg)
                nc.vector.tensor_reduce(
                    out=msq[:],
                    in_=sq[:].rearrange("p (q u) -> p q u", q=nq),
                    axis=mybir.AxisListType.X,
                    op=mybir.AluOpType.add)
                mag = stats_pool.tile([nr, nq], mybir.dt.float32,
                                      tag="mag" + tg, name="mag" + tg)
                nc.scalar.activation(out=mag[:], in_=msq[:],
                                     func=mybir.ActivationFunctionType.Sqrt)
                # fac = mag * 1/(1 + msq)  (== msq/((1+msq)sqrt(msq)))
                t1 = stats_pool.tile([nr, nq], mybir.dt.float32,
                                     tag="t1" + tg, name="t1" + tg)
                nc.vector.tensor_scalar_add(t1[:], msq[:], 1.0)
                rec = stats_pool.tile([nr, nq], mybir.dt.float32,
                                      tag="rec" + tg, name="rec" + tg)
                nc.vector.reciprocal(rec[:], t1[:])
                fac = stats_pool.tile([nr, nq], mybir.dt.float32,
                                      tag="fac" + tg, name="fac" + tg)
                nc.vector.tensor_mul(fac[:], mag[:], rec[:])
                fap = fac[:]
                fb = bass.AP(tensor=fap.tensor, offset=fap.offset,
                             ap=[fap.ap[0], fap.ap[1], [0, 128]])
                nc.vector.tensor_tensor(
                    out_ap.rearrange("p (q u) -> p q u", q=nq),
                    ps_ap.rearrange("p (q u) -> p q u", q=nq),
                    fb, mybir.AluOpType.mult)

            def finish(b):
                squash(state.pop(b)[:], NBANK, NQ, outt[:, b, :], "")

            for b in range(PB - 1):
                if b == 0:
                    # Ramp shaping. A DMA trigger occupies its engine
                    # ~0.6-1 us per MiB of descriptor-gen, so tiny first
                    # chunks get the first bytes moving sooner; h1's first
                    # 256 KiB rides the SP ring because the ACT ring's
                    # first descriptors queue behind its activation-table
                    # load (~1.3 us) and would otherwise idle the stream.
                    th = [acc_pool.tile([128, F], mm_dt, tag="acc",
                                        name=f"t{b}h{h}")
                          for h in range(HALVES)]
                    parts = [(0, 0, 512), (1, 0, 512), (1, 512, 1536),
                             (0, 512, 1536), (1, 1536, 2560),
                             (0, 1536, 2560), (1, 2560, F), (0, 2560, F)]
                    for i, (h, c0, c1) in enumerate(parts):
                        ring = rings[0] if (h == 0 or i == 1) else rings[1]
                        ring.dma_start(out=th[h][:, c0:c1],
                                       in_=x[b, h, :, c0:c1])
                else:
                    th = load_full(b)
                if b >= 1:
                    finish(b - 1)
                mms(b, th)

            # Last batch: fine-grained chunks on both rings. PSUM splits in
            # two row-groups; group A's data (chunks k0-3, both halves) is
            # loaded FIRST so its whole squash hides under group B's loads.
            # B's final 512-col chunk is split by column-halves so the last
            # squash runs as two pipelined half-chains.
            b = PB - 1
            th = [acc_pool.tile([128, F], mm_dt, tag="acc", name=f"t{b}h{h}")
                  for h in range(HALVES)]
            pA = [(h, k * 512, (k + 1) * 512)
                  for h in range(HALVES) for k in range(4)]
            pB = [(h, k * 512, (k + 1) * 512)
                  for h in range(HALVES) for k in range(4, NBANK)]
            load_tail_chunks(b, th, pA)
            finish(b - 1)
            load_tail_chunks(b, th, pB, i0=1)
            # Batches 0-6 stored early: the HBM-write receipt hides under
            # the remaining loads instead of the kernel tail.
            nc.sync.dma_start(out=y[:, :PB - 1], in_=outt[:, :PB - 1])

            psA = psum_pool.tile([4, 512], mybir.dt.float32, tag="psA",
                                 bufs=1)
            psB = psum_pool.tile([4, 512], mybir.dt.float32, tag="psB",
                                 bufs=1)
            for h in range(HALVES):
                for k in range(4):
                    nc.tensor.matmul(
                        psA[:, :], sel[:, k, 0:4],
                        th[h][:, k * 512:(k + 1) * 512],
                        start=(h == 0 and k == 0),
                        stop=(h == HALVES - 1 and k == 3))
            for h in range(HALVES):
                for k in range(4, NBANK):
                    nc.tensor.matmul(
                        psB[:, :], sel[:, k, 4:8],
                        th[h][:, k * 512:(k + 1) * 512],
                        start=(h == 0 and k == 4),
                        stop=(h == HALVES - 1 and k == NBANK - 1))
            squash(psA[:], 4, NQ, outt[:4, b, :], "A")
            nc.sync.dma_start(out=y[:4, b], in_=outt[:4, b])
            outb = out_pool.tile([4, 512], mybir.dt.float32)
            squash(psB[:], 4, NQ, outb[:], "B")
            nc.sync.dma_start(out=y[4:, b], in_=outb[:])

    nc.compile()
    return nc


_NC_CACHE = {}


def _get_nc():
    if "nc" not in _NC_CACHE:
        _NC_CACHE["nc"] = build_bass()
    return _NC_CACHE["nc"]


def kernel(x, **run_kwargs):
    x = np.ascontiguousarray(np.asarray(x, dtype=np.float32))
    assert x.shape == (B, NU, IC, US), x.shape

    nc = _get_nc()
    xs = x.reshape(N_CORES, PB, HALVES, 128, F)
    w = np.zeros((128, NBANK, NBANK), dtype=np.float32)
    for k in range(NBANK):
        w[:, k, k] = 1.0 / IC
    wb = w.astype(ml_dtypes.bfloat16)
    in_maps = [{"x": np.ascontiguousarray(xs[c]).astype(ml_dtypes.bfloat16),
                "w": wb}
               for c in range(N_CORES)]
    res = run_bass_kernel_spmd(nc, in_maps, core_ids=list(range(N_CORES)),
                               **run_kwargs)
    # y is (NBANK, PB, 512) per core; batch-major reshape on the host.
    out = np.stack([r["y"].transpose(1, 0, 2) for r in res.results], axis=0)
    out = out.reshape(B, NU, US, 1)
    if run_kwargs:
        kernel.last_results = res
    return out
